# revision 38
# baseline (speedup 1.0000x reference)
"""MHA kernel for trn2: B=4, T=2048, D=2048, NH=16, HD=128, causal, no scale.

Sharding: 8 cores = 4 batches x 2 head-groups (8 heads each core).

v3 pipeline (f16 matmul operands, fp32 PSUM accumulation):
  1. V projection token-major upfront: V[tok, (head, hd)] = x @ Wv via
     x^T chunks stationary / Wv moving (no PE transposes).
  2. Per head h: Q^T/K^T projection (W stationary, x^T moving), then
     K-major causal attention: S^T = K^T.T @ Q^T per 128-token s-chunk,
     E = exp(S^T) on ACT (f16 out, no max subtraction - logits small),
     diagonal chunks masked via f16 multiply, l partial sums on DVE/
     gpsimd (f32 += f16), O^T_unnorm accumulated in PSUM.
  3. Head h+1's projection matmuls are interleaved ~4 per attention
     s-chunk so the PE never stalls on ACT exp latency.
  Normalization (o_un / l) happens on host in fp64.
"""
import sys

sys.path.insert(0, '/opt/trn_rl_repo')

import numpy as np
import ml_dtypes
import concourse.bass as bass
import concourse.mybir as mybir
import concourse.tile as tile
from concourse import bacc, bass_utils

B, T, D = 4, 2048, 2048
NH, HD = 16, 128
HG = 2                      # head groups (tensor-parallel dim)
H_PER = NH // HG            # 8 heads per core
KO = D // 128               # 16 contraction chunks
TT = T // 512               # 4 t-tiles
SC = T // 128               # 16 s-chunks
TC = T // 128               # 16 token chunks (V projection)

f32 = mybir.dt.float32
f16 = mybir.dt.float16
f16np = np.float16
bf16 = mybir.dt.bfloat16
bf16np = ml_dtypes.bfloat16

# diagonal chunk k (s0 = t0 + 128k): compute columns [j0, j0+w) of the
# t-tile; with j0 = 128k every chunk's mask is plain i <= j (m0)
DIAG_W = [512, 384, 256, 128]
DIAG_J0 = [0, 128, 256, 384]
DIAG_MOFF = [0, 0, 0, 0]

_REPEAT = 1


def build_nc(repeat=1, bench_mode=False):
    nc = bacc.Bacc("TRN2", target_bir_lowering=False, debug=False)
    kind = "Internal" if bench_mode else "ExternalInput"
    xt_d = nc.dram_tensor("xt", [128, KO, T], f16, kind=kind)
    wqk_d = nc.dram_tensor("wqk", [H_PER, 2, 128, KO, 128], f16, kind=kind)
    wv_d = nc.dram_tensor("wv", [128, KO, H_PER * 128], f16, kind=kind)
    mk_d = nc.dram_tensor("mk", [128, 512], bf16, kind=kind)
    o_d = nc.dram_tensor("o_un", [H_PER, 128, T], f32, kind="ExternalOutput")
    l_d = nc.dram_tensor("l_acc", [H_PER, 128, T], f32, kind="ExternalOutput")

    with tile.TileContext(nc) as tc:
        with tc.tile_pool(name="const", bufs=1) as cpool:
            mk_sb = cpool.tile([128, 512], bf16)
            nc.sync.dma_start(mk_sb[:], mk_d.ap())

            for _ in range(repeat):
                with tc.tile_pool(name="xsb", bufs=1) as xpool, \
                     tc.tile_pool(name="wvsb", bufs=1) as wvpool, \
                     tc.tile_pool(name="vsb", bufs=1) as vpool, \
                     tc.tile_pool(name="wsb", bufs=4) as wpool, \
                     tc.tile_pool(name="ksb", bufs=2) as kpool, \
                     tc.tile_pool(name="qt", bufs=8) as qpool, \
                     tc.tile_pool(name="esb", bufs=3) as epool, \
                     tc.tile_pool(name="etm", bufs=2) as etpool, \
                     tc.tile_pool(name="lsb", bufs=4) as lpool, \
                     tc.tile_pool(name="osb", bufs=2) as opool, \
                     tc.tile_pool(name="pps", bufs=2, space="PSUM") as ppool, \
                     tc.tile_pool(name="sps", bufs=2, space="PSUM") as sps, \
                     tc.tile_pool(name="ops", bufs=2, space="PSUM") as ops:
                    # ---------- input DMA ----------
                    # first head's q,k weights must not queue behind bulk
                    w_sbs = {}

                    def fetch_w(h, c):
                        wt = wpool.tile([128, KO, 128], f16, tag="w")
                        nc.sync.dma_start(wt[:], wqk_d.ap()[h, c])
                        w_sbs[(h, c)] = wt

                    wv_sb = wvpool.tile([128, KO, H_PER * 128], f16)
                    x_sb = xpool.tile([128, KO, T], f16)
                    # stream wv + x in matched ko-chunks so V-projection
                    # groups can begin as soon as early chunks land; head-0
                    # q,k weights ride behind the first chunk (needed only
                    # after the whole V phase)
                    # x rides the ACT hwdge queue, wv/w the SP queue: two
                    # parallel input streams during the V phase
                    qs = [nc.sync, nc.scalar]

                    # first wave group needs only x[:,0,0:128] + wv[:,0,0:512];
                    # issue those as separate small DMAs so it starts sooner
                    nc.sync.dma_start(wv_sb[:, 0, 0:512],
                                      wv_d.ap()[:, 0, 0:512])
                    nc.scalar.dma_start(x_sb[:, 0, 0:128],
                                        xt_d.ap()[:, 0, 0:128])
                    nc.sync.dma_start(wv_sb[:, 0, 512:1024],
                                      wv_d.ap()[:, 0, 512:1024])
                    nc.scalar.dma_start(x_sb[:, 0, 128:512],
                                        xt_d.ap()[:, 0, 128:512])
                    for ko in range(1, KO):
                        qs[ko % 2].dma_start(wv_sb[:, ko], wv_d.ap()[:, ko])
                        qs[(ko + 1) % 2].dma_start(
                            x_sb[:, ko, 0:512], xt_d.ap()[:, ko, 0:512])
                        if ko == 2:
                            fetch_w(0, 0)
                            fetch_w(0, 1)
                    for tb in range(1, TT):
                        for ko in range(KO):
                            qs[ko % 2].dma_start(
                                x_sb[:, ko, tb * 512:(tb + 1) * 512],
                                xt_d.ap()[:, ko, tb * 512:(tb + 1) * 512])

                    # ---------- V projection (token-major, all heads) ----------
                    v_sb = vpool.tile([128, SC, H_PER * 128], bf16)
                    # block 0 runs ko-major waves across all 8 PSUM banks so
                    # each wave consumes exactly the ko-th (wv, x) DMA pair:
                    # the PE paces with the arriving stream instead of
                    # stalling for all 16 chunks of group 0
                    wave = []
                    for g in range(4):
                        spw = sps.tile([128, 1024], f32, tag="sp",
                                       name=f"vwave{g}")
                        wave.append((spw, 0))
                        wave.append((spw, 512))
                    for g in range(2):
                        wave.append((ppool.tile([128, 512], f32, tag="p",
                                                name=f"vwavep{g}"), 0))
                    for g in range(2):
                        wave.append((ops.tile([128, 512], f32, tag="op",
                                              name=f"vwaveo{g}"), 0))
                    for ko in range(KO):
                        for g in range(8):
                            tc_i, half = g // 2, g % 2
                            buf, off = wave[g]
                            nc.tensor.matmul(
                                buf[:, off:off + 512],
                                x_sb[:, ko, tc_i * 128:(tc_i + 1) * 128],
                                wv_sb[:, ko, half * 512:(half + 1) * 512],
                                start=(ko == 0), stop=(ko == KO - 1))
                    for g in range(8):
                        tc_i, half = g // 2, g % 2
                        buf, off = wave[g]
                        nc.vector.tensor_copy(
                            v_sb[:, tc_i, half * 512:(half + 1) * 512],
                            buf[:, off:off + 512])
                    for tc_i in range(4, TC):
                        for half in range(2):
                            vp = ppool.tile([128, 512], f32, tag="p")
                            for ko in range(KO):
                                nc.tensor.matmul(
                                    vp[:],
                                    x_sb[:, ko, tc_i * 128:(tc_i + 1) * 128],
                                    wv_sb[:, ko, half * 512:(half + 1) * 512],
                                    start=(ko == 0), stop=(ko == KO - 1))
                            nc.vector.tensor_copy(
                                v_sb[:, tc_i, half * 512:(half + 1) * 512],
                                vp[:])

                    # ---------- per-head pipeline ----------
                    def proj_gen(h, k_sb, q_ts):
                        """Q/K projection of head h as ~40 pump units (PE
                        quarters + copies). Prefetches head h+1 weights."""
                        if h + 1 < H_PER:
                            fetch_w(h + 1, 0)
                            fetch_w(h + 1, 1)
                        for c in range(2):
                            w_sb = w_sbs.pop((h, c))
                            for t in range(TT):
                                pt = ppool.tile([128, 512], f32, tag="p")
                                for ko4 in range(4):
                                    for koq in range(4):
                                        ko = ko4 * 4 + koq
                                        nc.tensor.matmul(
                                            pt[:], w_sb[:, ko],
                                            x_sb[:, ko, t * 512:(t + 1) * 512],
                                            start=(ko == 0),
                                            stop=(ko == KO - 1))
                                    yield 1
                                if c == 0:
                                    q_t = qpool.tile([128, 512], f16, tag="q")
                                    nc.vector.tensor_copy(q_t[:], pt[:])
                                    q_ts.append(q_t)
                                else:
                                    nc.vector.tensor_copy(
                                        k_sb[:, t * 512:(t + 1) * 512], pt[:])
                                yield 1

                    def attn(h, k_sb, q_ts, gen, t_order=tuple(range(TT)),
                             pump_cap=None, self_gen=None):
                        """Attention for head h, pumping `gen` (next head's
                        projection) ~1 unit per s-chunk. `self_gen` is the
                        un-pumped remainder of head h's own projection
                        (K tiles not needed until later t-tiles) used as
                        early filler when there is no next head."""
                        pumped = [0]

                        def pump(n=1):
                            for _ in range(n):
                                if self_gen is not None and \
                                        next(self_gen, None) is not None:
                                    continue
                                if gen is not None and (
                                        pump_cap is None
                                        or pumped[0] < pump_cap):
                                    pumped[0] += 1
                                    next(gen, None)

                        deferred = []
                        for t in t_order:
                            t0 = t * 512
                            n_chunks = 4 * (t + 1)
                            n_pairs = n_chunks // 2
                            op = ops.tile([128, 512], f32, tag="op")
                            l0 = lpool.tile([128, 512], f32, tag="l0")
                            l1 = lpool.tile([128, 512], f32, tag="l1")
                            e_info = []

                            def emit_pv(idx, op=op, e_info=e_info,
                                        n_chunks=n_chunks):
                                et_, base_, j0_, w_ = e_info[idx]
                                nc.tensor.matmul(
                                    op[:, j0_:j0_ + w_],
                                    v_sb[:, idx, h * 128:(h + 1) * 128],
                                    et_[:, base_:base_ + w_],
                                    start=(idx == 0), stop=(idx == n_chunks - 1))

                            def geom(s):
                                k_diag = s - 4 * t
                                if k_diag >= 0:
                                    return (DIAG_W[k_diag], DIAG_J0[k_diag],
                                            DIAG_MOFF[k_diag], True)
                                return 512, 0, 0, False

                            # chunks processed in pairs sharing one 2-bank
                            # PSUM tile and one exp activation
                            for p in range(n_pairs):
                                sa, sb = 2 * p, 2 * p + 1
                                wa, j0a, moffa, diag_a = geom(sa)
                                wb, j0b, moffb, diag_b = geom(sb)
                                sp = sps.tile([128, 1024], f32, tag="sp")
                                nc.tensor.matmul(
                                    sp[:, 0:wa],
                                    k_sb[:, sa * 128:(sa + 1) * 128],
                                    q_ts[t][:, j0a:j0a + wa],
                                    start=True, stop=True)
                                pump()
                                nc.tensor.matmul(
                                    sp[:, 512:512 + wb],
                                    k_sb[:, sb * 128:(sb + 1) * 128],
                                    q_ts[t][:, j0b:j0b + wb],
                                    start=True, stop=True)
                                pump()
                                et = epool.tile([128, 1024], bf16, tag="e")
                                span = 512 + wb
                                if diag_a or diag_b:
                                    etmp = etpool.tile([128, 1024], bf16,
                                                       tag="etmp")
                                    nc.scalar.activation(
                                        etmp[:, 0:span], sp[:, 0:span],
                                        mybir.ActivationFunctionType.Exp)
                                    nc.vector.tensor_tensor(
                                        et[:, 0:wa], etmp[:, 0:wa],
                                        mk_sb[:, moffa:moffa + wa],
                                        mybir.AluOpType.mult)
                                    nc.vector.tensor_tensor(
                                        et[:, 512:512 + wb],
                                        etmp[:, 512:512 + wb],
                                        mk_sb[:, moffb:moffb + wb],
                                        mybir.AluOpType.mult)
                                else:
                                    nc.scalar.activation(
                                        et[:, 0:span], sp[:, 0:span],
                                        mybir.ActivationFunctionType.Exp)
                                e_info.append((et, 0, j0a, wa))
                                e_info.append((et, 512, j0b, wb))
                                # l partial sums split across gpsimd and DVE
                                for s, base, w, j0 in ((sa, 0, wa, j0a),
                                                       (sb, 512, wb, j0b)):
                                    eng = nc.gpsimd if s % 2 else nc.vector
                                    lx = l1 if s % 2 else l0
                                    if s == 0 or (s == 1 and t > 0):
                                        eng.tensor_copy(
                                            lx[:], et[:, base:base + 512])
                                    else:
                                        if s == 1:  # t == 0: truncated l1
                                            nc.gpsimd.memset(lx[:], 0.0)
                                        eng.tensor_tensor(
                                            lx[:, j0:j0 + w], lx[:, j0:j0 + w],
                                            et[:, base:base + w],
                                            mybir.AluOpType.add)
                                if p >= 1:
                                    emit_pv(2 * p - 2)
                                    emit_pv(2 * p - 1)
                                # previous tile's tail rides here, after this
                                # tile's first S-matmuls hide its exp wait
                                if p == 1 and deferred:
                                    for f in deferred:
                                        f()
                                    deferred = []

                            def tile_tail(op=op, l0=l0, l1=l1, t0=t0,
                                          n_chunks=n_chunks, emit_pv=emit_pv):
                                nc.vector.tensor_tensor(
                                    l0[:], l0[:], l1[:], mybir.AluOpType.add)
                                emit_pv(n_chunks - 2)
                                emit_pv(n_chunks - 1)
                                o_sb = opool.tile([128, 512], f32, tag="o")
                                nc.vector.tensor_copy(o_sb[:], op[:])
                                nc.sync.dma_start(
                                    o_d.ap()[h, :, t0:t0 + 512], o_sb[:])
                                nc.sync.dma_start(
                                    l_d.ap()[h, :, t0:t0 + 512], l0[:])
                            deferred = [tile_tail]
                        for f in deferred:
                            f()
                        # drain leftovers (unless capped for handoff)
                        if self_gen is not None:
                            for _ in self_gen:
                                pass
                        if gen is not None and pump_cap is None:
                            for _ in gen:
                                pass

                    # head 0 projection runs standalone after the V phase
                    k_cur = kpool.tile([128, T], f16, tag="k")
                    q_cur = []
                    for _ in proj_gen(0, k_cur, q_cur):
                        pass
                    for h in range(H_PER):
                        if h + 1 < H_PER:
                            k_nxt = kpool.tile([128, T], f16, tag="k")
                            q_nxt = []
                            gen = proj_gen(h + 1, k_nxt, q_nxt)
                        else:
                            k_nxt, q_nxt, gen = None, None, None
                        if h == H_PER - 2:
                            # leave head 7's K tiles 2,3 (10 units) unpumped:
                            # they become attn(7)'s early filler, since no
                            # next head exists to hide its exp latency
                            attn(h, k_cur, q_cur, gen, pump_cap=25)
                            carry = gen
                        elif h == H_PER - 1:
                            # end on the smallest tile so the final output
                            # flush has the least work after the last matmul
                            attn(h, k_cur, q_cur, None, self_gen=carry,
                                 t_order=(0, 2, 3, 1))
                        else:
                            attn(h, k_cur, q_cur, gen)
                        k_cur, q_cur = k_nxt, q_nxt
    nc.compile()
    return nc


def _host_prep(x, qkv_proj):
    """Build per-core input maps. Cores: c -> (b = c // 2, hg = c % 2)."""
    xts = []
    for b in range(B):
        xt = np.ascontiguousarray(x[b].T)             # [D, T]
        xts.append(np.ascontiguousarray(
            xt.reshape(KO, 128, T).transpose(1, 0, 2)).astype(f16np))
    wqks, wvs = [], []
    for hg in range(HG):
        wqk = np.empty((H_PER, 2, 128, KO, 128), np.float32)
        for h in range(H_PER):
            for c in range(2):
                w = qkv_proj[c, hg * (H_PER * HD) + h * HD:
                             hg * (H_PER * HD) + (h + 1) * HD, :]   # [128, D]
                wqk[h, c] = w.T.reshape(KO, 128, 128).transpose(1, 0, 2)
        wqks.append(wqk.astype(f16np))
        wv = qkv_proj[2, hg * (H_PER * HD):(hg + 1) * (H_PER * HD), :]
        wv = wv.T.reshape(KO, 128, H_PER * 128).transpose(1, 0, 2)
        wvs.append(np.ascontiguousarray(wv).astype(f16np))
    # causal mask m0[i, j] = (i <= j)
    ii = np.arange(128)[:, None]
    mk = (ii <= np.arange(512)[None, :]).astype(np.float32)
    mk = mk.astype(bf16np)
    in_maps = []
    for c in range(8):
        b, hg = c // 2, c % 2
        in_maps.append({"xt": xts[b], "wqk": wqks[hg], "wv": wvs[hg],
                        "mk": mk})
    return in_maps


def _assemble(results):
    out = np.empty((B, T, NH * HD), np.float32)
    for c in range(8):
        b, hg = c // 2, c % 2
        o_un = results[c]["o_un"].astype(np.float64)       # [H_PER, 128, T]
        l_sum = results[c]["l_acc"].astype(np.float64)     # [H_PER, 128, T]
        l_sum = l_sum.sum(axis=1)                          # [H_PER, T]
        o = o_un / l_sum[:, None, :]
        out[b, :, hg * (H_PER * HD):(hg + 1) * (H_PER * HD)] = (
            o.transpose(2, 0, 1).reshape(T, H_PER * HD))
    return out


_NC_CACHE = {}


def _get_nc(repeat=1):
    if repeat not in _NC_CACHE:
        _NC_CACHE[repeat] = build_nc(repeat)
    return _NC_CACHE[repeat]


def kernel(x, qkv_proj):
    x = np.asarray(x, np.float32)
    qkv_proj = np.asarray(qkv_proj, np.float32)
    nc = _get_nc(_REPEAT)
    in_maps = _host_prep(x, qkv_proj)
    res = bass_utils.run_bass_kernel_spmd(nc, in_maps, core_ids=list(range(8)))
    return _assemble(res.results)


# revision 42
# speedup vs baseline: 1.1262x; 1.1262x over previous
"""MHA kernel for trn2: B=4, T=2048, D=2048, NH=16, HD=128, causal, no scale.

Sharding: 8 cores = 4 batches x 2 head-groups (8 heads each core).

v3 pipeline (f16 matmul operands, fp32 PSUM accumulation):
  1. V projection token-major upfront: V[tok, (head, hd)] = x @ Wv via
     x^T chunks stationary / Wv moving (no PE transposes).
  2. Per head h: Q^T/K^T projection (W stationary, x^T moving), then
     K-major causal attention: S^T = K^T.T @ Q^T per 128-token s-chunk,
     E = exp(S^T) on ACT (f16 out, no max subtraction - logits small),
     diagonal chunks masked via f16 multiply, l partial sums on DVE/
     gpsimd (f32 += f16), O^T_unnorm accumulated in PSUM.
  3. Head h+1's projection matmuls are interleaved ~4 per attention
     s-chunk so the PE never stalls on ACT exp latency. The last head
     carries its own not-yet-needed K-tile projections (capped pumping
     during head 6) as early filler, since no next head follows it.
  Normalization (o_un / l) happens on host in fp64.
"""
import sys

sys.path.insert(0, '/opt/trn_rl_repo')

import numpy as np
import ml_dtypes
import concourse.bass as bass
import concourse.mybir as mybir
import concourse.tile as tile
from concourse import bacc, bass_utils

B, T, D = 4, 2048, 2048
NH, HD = 16, 128
HG = 2                      # head groups (tensor-parallel dim)
H_PER = NH // HG            # 8 heads per core
KO = D // 128               # 16 contraction chunks
TT = T // 512               # 4 t-tiles
SC = T // 128               # 16 s-chunks
TC = T // 128               # 16 token chunks (V projection)

f32 = mybir.dt.float32
f16 = mybir.dt.float16
f16np = np.float16
bf16 = mybir.dt.bfloat16
bf16np = ml_dtypes.bfloat16

# diagonal chunk k (s0 = t0 + 128k): compute columns [j0, j0+w) of the
# t-tile; with j0 = 128k every chunk's mask is plain i <= j (m0)
DIAG_W = [512, 384, 256, 128]
DIAG_J0 = [0, 128, 256, 384]
DIAG_MOFF = [0, 0, 0, 0]

_REPEAT = 1


def build_nc(repeat=1, bench_mode=False):
    nc = bacc.Bacc("TRN2", target_bir_lowering=False, debug=False)
    kind = "Internal" if bench_mode else "ExternalInput"
    xt_d = nc.dram_tensor("xt", [128, KO, T], f16, kind=kind)
    wqk_d = nc.dram_tensor("wqk", [H_PER, 2, 128, KO, 128], f16, kind=kind)
    wv_d = nc.dram_tensor("wv", [128, KO, H_PER * 128], f16, kind=kind)
    mk_d = nc.dram_tensor("mk", [128, 512], bf16, kind=kind)
    o_d = nc.dram_tensor("o_un", [H_PER, 128, T], f32, kind="ExternalOutput")
    l_d = nc.dram_tensor("l_acc", [H_PER, 128, T], f32, kind="ExternalOutput")

    with tile.TileContext(nc) as tc:
        with tc.tile_pool(name="const", bufs=1) as cpool:
            mk_sb = cpool.tile([128, 512], bf16)
            nc.sync.dma_start(mk_sb[:], mk_d.ap())

            for _ in range(repeat):
                with tc.tile_pool(name="xsb", bufs=1) as xpool, \
                     tc.tile_pool(name="wvsb", bufs=1) as wvpool, \
                     tc.tile_pool(name="vsb", bufs=1) as vpool, \
                     tc.tile_pool(name="wsb", bufs=5) as wpool, \
                     tc.tile_pool(name="ksb", bufs=2) as kpool, \
                     tc.tile_pool(name="qt", bufs=9) as qpool, \
                     tc.tile_pool(name="esb", bufs=4) as epool, \
                     tc.tile_pool(name="etm", bufs=3) as etpool, \
                     tc.tile_pool(name="lsb", bufs=5) as lpool, \
                     tc.tile_pool(name="osb", bufs=2) as opool, \
                     tc.tile_pool(name="pps", bufs=2, space="PSUM") as ppool, \
                     tc.tile_pool(name="sps", bufs=2, space="PSUM") as sps, \
                     tc.tile_pool(name="ops", bufs=2, space="PSUM") as ops:
                    # ---------- input DMA ----------
                    # first head's q,k weights must not queue behind bulk
                    w_sbs = {}

                    def fetch_w(h, c):
                        wt = wpool.tile([128, KO, 128], f16, tag="w")
                        nc.sync.dma_start(wt[:], wqk_d.ap()[h, c])
                        w_sbs[(h, c)] = wt

                    wv_sb = wvpool.tile([128, KO, H_PER * 128], f16)
                    x_sb = xpool.tile([128, KO, T], f16)
                    # stream wv + x in matched ko-chunks so V-projection
                    # groups can begin as soon as early chunks land; head-0
                    # q,k weights ride behind the first chunk (needed only
                    # after the whole V phase)
                    # x rides the ACT hwdge queue, wv/w the SP queue: two
                    # parallel input streams during the V phase
                    qs = [nc.sync, nc.scalar]

                    # first wave group needs only x[:,0,0:128] + wv[:,0,0:512];
                    # issue those as separate small DMAs so it starts sooner
                    nc.sync.dma_start(wv_sb[:, 0, 0:512],
                                      wv_d.ap()[:, 0, 0:512])
                    nc.scalar.dma_start(x_sb[:, 0, 0:128],
                                        xt_d.ap()[:, 0, 0:128])
                    nc.sync.dma_start(wv_sb[:, 0, 512:1024],
                                      wv_d.ap()[:, 0, 512:1024])
                    nc.scalar.dma_start(x_sb[:, 0, 128:512],
                                        xt_d.ap()[:, 0, 128:512])
                    for ko in range(1, KO):
                        qs[ko % 2].dma_start(wv_sb[:, ko], wv_d.ap()[:, ko])
                        qs[(ko + 1) % 2].dma_start(
                            x_sb[:, ko, 0:512], xt_d.ap()[:, ko, 0:512])
                        if ko == 2:
                            fetch_w(0, 0)
                            fetch_w(0, 1)
                    for tb in range(1, TT):
                        for ko in range(KO):
                            qs[ko % 2].dma_start(
                                x_sb[:, ko, tb * 512:(tb + 1) * 512],
                                xt_d.ap()[:, ko, tb * 512:(tb + 1) * 512])

                    # ---------- V projection (token-major, all heads) ----------
                    v_sb = vpool.tile([128, SC, H_PER * 128], bf16)
                    # block 0 runs ko-major waves across all 8 PSUM banks so
                    # each wave consumes exactly the ko-th (wv, x) DMA pair:
                    # the PE paces with the arriving stream instead of
                    # stalling for all 16 chunks of group 0
                    wave = []
                    for g in range(4):
                        spw = sps.tile([128, 1024], f32, tag="sp",
                                       name=f"vwave{g}")
                        wave.append((spw, 0))
                        wave.append((spw, 512))
                    for g in range(2):
                        wave.append((ppool.tile([128, 512], f32, tag="p",
                                                name=f"vwavep{g}"), 0))
                    for g in range(2):
                        wave.append((ops.tile([128, 512], f32, tag="op",
                                              name=f"vwaveo{g}"), 0))
                    for ko in range(KO):
                        for g in range(8):
                            tc_i, half = g // 2, g % 2
                            buf, off = wave[g]
                            nc.tensor.matmul(
                                buf[:, off:off + 512],
                                x_sb[:, ko, tc_i * 128:(tc_i + 1) * 128],
                                wv_sb[:, ko, half * 512:(half + 1) * 512],
                                start=(ko == 0), stop=(ko == KO - 1))
                    for g in range(8):
                        tc_i, half = g // 2, g % 2
                        buf, off = wave[g]
                        nc.vector.tensor_copy(
                            v_sb[:, tc_i, half * 512:(half + 1) * 512],
                            buf[:, off:off + 512])
                    for tc_i in range(4, TC):
                        for half in range(2):
                            vp = ppool.tile([128, 512], f32, tag="p")
                            for ko in range(KO):
                                nc.tensor.matmul(
                                    vp[:],
                                    x_sb[:, ko, tc_i * 128:(tc_i + 1) * 128],
                                    wv_sb[:, ko, half * 512:(half + 1) * 512],
                                    start=(ko == 0), stop=(ko == KO - 1))
                            nc.vector.tensor_copy(
                                v_sb[:, tc_i, half * 512:(half + 1) * 512],
                                vp[:])

                    # ---------- per-head pipeline ----------
                    # interleaved group order: k t0 lands mid-way through
                    # the previous head's attention, not in its very last
                    # pump slot right before the next head reads it
                    PROJ_ORDER = ((0, 0), (1, 0), (0, 1), (1, 1),
                                  (0, 2), (1, 2), (0, 3), (1, 3))

                    def proj_gen(h, k_sb, q_ts, order=PROJ_ORDER):
                        """Q/K projection of head h as ~40 pump units (PE
                        quarters + copies). Prefetches head h+1 weights."""
                        if h + 1 < H_PER:
                            fetch_w(h + 1, 0)
                            fetch_w(h + 1, 1)
                        w_cs = [w_sbs.pop((h, 0)), w_sbs.pop((h, 1))]
                        for c, t in order:
                            w_sb = w_cs[c]
                            pt = ppool.tile([128, 512], f32, tag="p")
                            for ko4 in range(4):
                                for koq in range(4):
                                    ko = ko4 * 4 + koq
                                    nc.tensor.matmul(
                                        pt[:], w_sb[:, ko],
                                        x_sb[:, ko, t * 512:(t + 1) * 512],
                                        start=(ko == 0),
                                        stop=(ko == KO - 1))
                                yield 1
                            if c == 0:
                                q_t = qpool.tile([128, 512], f16, tag="q")
                                nc.vector.tensor_copy(q_t[:], pt[:])
                                q_ts.append(q_t)
                            else:
                                nc.vector.tensor_copy(
                                    k_sb[:, t * 512:(t + 1) * 512], pt[:])
                            yield 1

                    def attn(h, k_sb, q_ts, gen, t_order=tuple(range(TT)),
                             pump_cap=None, self_gen=None):
                        """Attention for head h, pumping `gen` (next head's
                        projection) ~1 unit per s-chunk. `self_gen` is the
                        un-pumped remainder of head h's own projection
                        (K tiles not needed until later t-tiles) used as
                        early filler when there is no next head."""
                        pumped = [0]

                        def pump(n=1):
                            for _ in range(n):
                                if self_gen is not None and \
                                        next(self_gen, None) is not None:
                                    continue
                                if gen is not None and (
                                        pump_cap is None
                                        or pumped[0] < pump_cap):
                                    pumped[0] += 1
                                    next(gen, None)

                        deferred = []
                        for t in t_order:
                            t0 = t * 512
                            n_chunks = 4 * (t + 1)
                            n_pairs = n_chunks // 2
                            op = ops.tile([128, 512], f32, tag="op")
                            l0 = lpool.tile([128, 512], f32, tag="l0")
                            l1 = lpool.tile([128, 512], f32, tag="l1")
                            e_info = []

                            def emit_pv(idx, op=op, e_info=e_info,
                                        n_chunks=n_chunks):
                                et_, base_, j0_, w_ = e_info[idx]
                                nc.tensor.matmul(
                                    op[:, j0_:j0_ + w_],
                                    v_sb[:, idx, h * 128:(h + 1) * 128],
                                    et_[:, base_:base_ + w_],
                                    start=(idx == 0), stop=(idx == n_chunks - 1))

                            def geom(s):
                                k_diag = s - 4 * t
                                if k_diag >= 0:
                                    return (DIAG_W[k_diag], DIAG_J0[k_diag],
                                            DIAG_MOFF[k_diag], True)
                                return 512, 0, 0, False

                            # chunks processed in pairs sharing one 2-bank
                            # PSUM tile and one exp activation
                            for p in range(n_pairs):
                                sa, sb = 2 * p, 2 * p + 1
                                wa, j0a, moffa, diag_a = geom(sa)
                                wb, j0b, moffb, diag_b = geom(sb)
                                sp = sps.tile([128, 1024], f32, tag="sp")
                                nc.tensor.matmul(
                                    sp[:, 0:wa],
                                    k_sb[:, sa * 128:(sa + 1) * 128],
                                    q_ts[t][:, j0a:j0a + wa],
                                    start=True, stop=True)
                                pump()
                                nc.tensor.matmul(
                                    sp[:, 512:512 + wb],
                                    k_sb[:, sb * 128:(sb + 1) * 128],
                                    q_ts[t][:, j0b:j0b + wb],
                                    start=True, stop=True)
                                pump()
                                et = epool.tile([128, 1024], bf16, tag="e")
                                span = 512 + wb
                                if diag_a or diag_b:
                                    etmp = etpool.tile([128, 1024], bf16,
                                                       tag="etmp")
                                    nc.scalar.activation(
                                        etmp[:, 0:span], sp[:, 0:span],
                                        mybir.ActivationFunctionType.Exp)
                                    nc.vector.tensor_tensor(
                                        et[:, 0:wa], etmp[:, 0:wa],
                                        mk_sb[:, moffa:moffa + wa],
                                        mybir.AluOpType.mult)
                                    nc.vector.tensor_tensor(
                                        et[:, 512:512 + wb],
                                        etmp[:, 512:512 + wb],
                                        mk_sb[:, moffb:moffb + wb],
                                        mybir.AluOpType.mult)
                                else:
                                    nc.scalar.activation(
                                        et[:, 0:span], sp[:, 0:span],
                                        mybir.ActivationFunctionType.Exp)
                                e_info.append((et, 0, j0a, wa))
                                e_info.append((et, 512, j0b, wb))
                                # l partial sums split across gpsimd and DVE
                                for s, base, w, j0 in ((sa, 0, wa, j0a),
                                                       (sb, 512, wb, j0b)):
                                    eng = nc.gpsimd if s % 2 else nc.vector
                                    lx = l1 if s % 2 else l0
                                    if s == 0 or (s == 1 and t > 0):
                                        eng.tensor_copy(
                                            lx[:], et[:, base:base + 512])
                                    else:
                                        if s == 1:  # t == 0: truncated l1
                                            nc.gpsimd.memset(lx[:], 0.0)
                                        eng.tensor_tensor(
                                            lx[:, j0:j0 + w], lx[:, j0:j0 + w],
                                            et[:, base:base + w],
                                            mybir.AluOpType.add)
                                if p >= 1:
                                    emit_pv(2 * p - 2)
                                    emit_pv(2 * p - 1)
                                # previous tile's tail rides here, after this
                                # tile's first S-matmuls hide its exp wait
                                if p == 1 and deferred:
                                    for f in deferred:
                                        f()
                                    deferred = []

                            def tile_tail(op=op, l0=l0, l1=l1, t0=t0,
                                          n_chunks=n_chunks, emit_pv=emit_pv):
                                nc.vector.tensor_tensor(
                                    l0[:], l0[:], l1[:], mybir.AluOpType.add)
                                emit_pv(n_chunks - 2)
                                emit_pv(n_chunks - 1)
                                o_sb = opool.tile([128, 512], f32, tag="o")
                                nc.vector.tensor_copy(o_sb[:], op[:])
                                nc.sync.dma_start(
                                    o_d.ap()[h, :, t0:t0 + 512], o_sb[:])
                                nc.sync.dma_start(
                                    l_d.ap()[h, :, t0:t0 + 512], l0[:])
                            deferred = [tile_tail]
                        for f in deferred:
                            f()
                        # drain leftovers (unless capped for handoff)
                        if self_gen is not None:
                            for _ in self_gen:
                                pass
                        if gen is not None and pump_cap is None:
                            for _ in gen:
                                pass

                    # head 0 projection runs standalone after the V phase
                    k_cur = kpool.tile([128, T], f16, tag="k")
                    q_cur = []
                    for _ in proj_gen(0, k_cur, q_cur):
                        pass
                    for h in range(H_PER):
                        if h + 1 < H_PER:
                            k_nxt = kpool.tile([128, T], f16, tag="k")
                            q_nxt = []
                            gen = proj_gen(h + 1, k_nxt, q_nxt)
                        else:
                            k_nxt, q_nxt, gen = None, None, None
                        if h == H_PER - 2:
                            # leave head 7's K tiles 2,3 (10 units) unpumped:
                            # they become attn(7)'s early filler, since no
                            # next head exists to hide its exp latency
                            attn(h, k_cur, q_cur, gen, pump_cap=25)
                            carry = gen
                        elif h == H_PER - 1:
                            # end on the smallest tile so the final output
                            # flush has the least work after the last matmul
                            attn(h, k_cur, q_cur, None, self_gen=carry,
                                 t_order=(0, 2, 3, 1))
                        else:
                            attn(h, k_cur, q_cur, gen)
                        k_cur, q_cur = k_nxt, q_nxt
    nc.compile()
    return nc


def _host_prep(x, qkv_proj):
    """Build per-core input maps. Cores: c -> (b = c // 2, hg = c % 2)."""
    xts = []
    for b in range(B):
        xt = np.ascontiguousarray(x[b].T)             # [D, T]
        xts.append(np.ascontiguousarray(
            xt.reshape(KO, 128, T).transpose(1, 0, 2)).astype(f16np))
    wqks, wvs = [], []
    for hg in range(HG):
        wqk = np.empty((H_PER, 2, 128, KO, 128), np.float32)
        for h in range(H_PER):
            for c in range(2):
                w = qkv_proj[c, hg * (H_PER * HD) + h * HD:
                             hg * (H_PER * HD) + (h + 1) * HD, :]   # [128, D]
                wqk[h, c] = w.T.reshape(KO, 128, 128).transpose(1, 0, 2)
        wqks.append(wqk.astype(f16np))
        wv = qkv_proj[2, hg * (H_PER * HD):(hg + 1) * (H_PER * HD), :]
        wv = wv.T.reshape(KO, 128, H_PER * 128).transpose(1, 0, 2)
        wvs.append(np.ascontiguousarray(wv).astype(f16np))
    # causal mask m0[i, j] = (i <= j)
    ii = np.arange(128)[:, None]
    mk = (ii <= np.arange(512)[None, :]).astype(np.float32)
    mk = mk.astype(bf16np)
    in_maps = []
    for c in range(8):
        b, hg = c // 2, c % 2
        in_maps.append({"xt": xts[b], "wqk": wqks[hg], "wv": wvs[hg],
                        "mk": mk})
    return in_maps


def _assemble(results):
    out = np.empty((B, T, NH * HD), np.float32)
    for c in range(8):
        b, hg = c // 2, c % 2
        o_un = results[c]["o_un"].astype(np.float64)       # [H_PER, 128, T]
        l_sum = results[c]["l_acc"].astype(np.float64)     # [H_PER, 128, T]
        l_sum = l_sum.sum(axis=1)                          # [H_PER, T]
        o = o_un / l_sum[:, None, :]
        out[b, :, hg * (H_PER * HD):(hg + 1) * (H_PER * HD)] = (
            o.transpose(2, 0, 1).reshape(T, H_PER * HD))
    return out


_NC_CACHE = {}


def _get_nc(repeat=1):
    if repeat not in _NC_CACHE:
        _NC_CACHE[repeat] = build_nc(repeat)
    return _NC_CACHE[repeat]


def kernel(x, qkv_proj):
    x = np.asarray(x, np.float32)
    qkv_proj = np.asarray(qkv_proj, np.float32)
    nc = _get_nc(_REPEAT)
    in_maps = _host_prep(x, qkv_proj)
    res = bass_utils.run_bass_kernel_spmd(nc, in_maps, core_ids=list(range(8)))
    return _assemble(res.results)
